# revision 14
# baseline (speedup 1.0000x reference)
"""Trainium2 Bass kernel for nn_DGLJTNNEncoder (JT-NN tree-GRU encoder).

Problem structure (hardcoded; matches reference._build_forest):
  256 perfect binary trees, depth 7 (255 nodes, 508 directed edges each).
  14 sequential BFS levels: bottom-up d=7..1 (up-edges), top-down d=1..7
  (down-edges).  Per level a GRU-style edge update; outputs the full edge
  message tensor m [130048, 256] and root_vecs [256, 256].

Sharding: data-parallel by tree — 32 whole trees per NeuronCore, weights
replicated, no cross-core communication (host concatenates outputs).

Device pipeline (per core, feature-major [feature, column] layout):
  - embedding table cast to bf16, rows gathered transposed via dma_gather
    into x [128, 2, 8192] (feature-major, tree-blocked columns)
  - per level: z/h/r gate matmuls in bf16 on PE accumulating fp32 in PSUM
    (K=512 contractions in 128-chunks, funky APs give tree/dup-2 column
    patterns for free), sigmoids/tanh on ACT, GRU combine on DVE,
    tree-structured pair-sums on GPSIMD/DVE
  - m written out via PE transpose (bf16 -> PSUM) + evac + strided DMA
    into the natural [edge, 256] fp32 layout
"""

import numpy as np
from contextlib import ExitStack

import concourse.bass as bass
import concourse.tile as tile
from concourse import bacc, mybir
from concourse import masks
from concourse.bass_utils import run_bass_kernel_spmd
from bass_rust import add_dep_helper

f32 = mybir.dt.float32
bf = mybir.dt.bfloat16
AF = mybir.ActivationFunctionType

D = 7
NPT = 255            # nodes per tree
EPT = 508            # edges per tree
T = 256              # total trees
NCORES = 8
TPC = T // NCORES    # 32 trees per core
NPC = TPC * NPT      # 8160 nodes per core
EPC = TPC * EPT      # 16256 edges per core
NIDX = 8192          # padded gather count (multiple of 128)
H = 256
V = 780
CH = 512             # column chunk size (one PSUM bank of fp32)

# level list: (is_bu, d). K = TPC * 2**d columns.
LEVELS = [(True, d) for d in range(D, 0, -1)] + [(False, d) for d in range(1, D + 1)]
# column offset of bottom-up level d inside the 8128-column BU arenas
BU_OFF = {}
_off = 0
for d in range(D, 0, -1):
    BU_OFF[d] = _off
    _off += TPC * (2 ** d)
BU_COLS = _off  # 8128


def build_program(repeat=1):
    """Build and compile the per-core Bass program. Returns nc."""
    nc = bacc.Bacc("TRN2", target_bir_lowering=False, debug=False)

    emb_d = nc.dram_tensor("emb", [V, H], f32, kind="ExternalInput").ap()
    wz_d = nc.dram_tensor("Wz_w", [2 * H, H], f32, kind="ExternalInput").ap()
    wh_d = nc.dram_tensor("Wh_w", [2 * H, H], f32, kind="ExternalInput").ap()
    wg_d = nc.dram_tensor("Wg_w", [2 * H, H], f32, kind="ExternalInput").ap()
    wr_d = nc.dram_tensor("Wr_w", [H, H], f32, kind="ExternalInput").ap()
    ur_d = nc.dram_tensor("Ur_w", [H, H], f32, kind="ExternalInput").ap()
    bz_d = nc.dram_tensor("Wz_b", [H], f32, kind="ExternalInput").ap()
    bh_d = nc.dram_tensor("Wh_b", [H], f32, kind="ExternalInput").ap()
    br_d = nc.dram_tensor("Ur_b", [H], f32, kind="ExternalInput").ap()
    bg_d = nc.dram_tensor("Wg_b", [H], f32, kind="ExternalInput").ap()
    wid_d = nc.dram_tensor("wid_pad", [NIDX], mybir.dt.int32, kind="ExternalInput").ap()

    m_out = nc.dram_tensor("m_out", [EPC, H], f32, kind="ExternalOutput").ap()
    root_out = nc.dram_tensor("root_out", [TPC, H], f32, kind="ExternalOutput").ap()

    emb_bf = nc.dram_tensor("emb_bf16", [V, H], bf).ap()

    def flat128(ap):
        return ap.rearrange("v h -> (v h)").rearrange("(p q) -> p q", p=128)

    # edge-row view of m_out: row of edge (tree, a, b) = t*508 + 2a + b
    mview = m_out.rearrange("(t a b) f -> t a b f", t=TPC, a=NPT - 1, b=2)

    with tile.TileContext(nc) as tc, ExitStack() as ctx:
        persist = ctx.enter_context(tc.tile_pool(name="persist", bufs=1))
        prolog = ctx.enter_context(tc.tile_pool(name="prolog", bufs=1))
        spool = ctx.enter_context(tc.tile_pool(name="spool", bufs=2))
        zpool = ctx.enter_context(tc.tile_pool(name="zpool", bufs=2))
        tpool = ctx.enter_context(tc.tile_pool(name="tpool", bufs=2))
        stg = ctx.enter_context(tc.tile_pool(name="stg", bufs=3))
        xpool = ctx.enter_context(tc.tile_pool(name="xpool", bufs=2))
        ps_z = ctx.enter_context(tc.tile_pool(name="ps_z", bufs=2, space="PSUM"))
        ps_h = ctx.enter_context(tc.tile_pool(name="ps_h", bufs=2, space="PSUM"))
        ps_r = ctx.enter_context(tc.tile_pool(name="ps_r", bufs=2, space="PSUM"))
        ps_t = ctx.enter_context(tc.tile_pool(name="ps_t", bufs=2, space="PSUM"))

        # ---------- persistent tiles ----------
        xt = persist.tile([128, 2, NIDX], bf)                   # x feature-major
        mbu = persist.tile([128, 2, BU_COLS], bf)
        rmbu = persist.tile([128, 2, BU_COLS], bf)
        mtd = {0: persist.tile([128, 2, TPC * 64], bf, tag="mtd0", name="mtd0"),   # d even (2,4,6)
               1: persist.tile([128, 2, TPC * 32], bf, tag="mtd1", name="mtd1")}   # d odd (1,3,5)
        rmtd = {0: persist.tile([128, 2, TPC * 64], bf, tag="rmtd0", name="rmtd0"),
                1: persist.tile([128, 2, TPC * 32], bf, tag="rmtd1", name="rmtd1")}
        wz_sb = persist.tile([128, 4, H], bf)
        wh_sb = persist.tile([128, 4, H], bf)
        wg_sb = persist.tile([128, 4, H], bf)
        wr_sb = persist.tile([128, 2, H], bf)
        ur_sb = persist.tile([128, 2, H], bf)
        bz_sb = persist.tile([128, 2], f32)
        bh_sb = persist.tile([128, 2], f32)
        br_sb = persist.tile([128, 2], f32)
        wgb_sb = persist.tile([1, H], bf)
        ones_sb = persist.tile([1, TPC], bf)
        idn = persist.tile([128, 128], bf)
        idx16 = persist.tile([128, NIDX // 16], mybir.dt.int16)

        # ---------- prologue ----------
        masks.make_identity(nc, idn[:])
        nc.gpsimd.memset(ones_sb[:], 1.0)

        # weights -> SBUF bf16, layout [p, kc, m]
        for w_d, w_sb, nkc in ((wz_d, wz_sb, 4), (wh_d, wh_sb, 4), (wg_d, wg_sb, 4),
                               (wr_d, wr_sb, 2), (ur_d, ur_sb, 2)):
            w32 = prolog.tile([128, nkc, H], f32, tag="w32")
            nc.sync.dma_start(w32[:], w_d.rearrange("(kc p) m -> p kc m", p=128))
            nc.vector.tensor_copy(w_sb[:], w32[:])
        for b_d, b_sb in ((bz_d, bz_sb), (bh_d, bh_sb), (br_d, br_sb)):
            nc.sync.dma_start(b_sb[:], b_d.rearrange("(mc p) -> p mc", p=128))
        bg32 = prolog.tile([1, H], f32, tag="bg")
        nc.sync.dma_start(bg32[:], bg_d.rearrange("(a h) -> a h", a=1))
        nc.vector.tensor_copy(wgb_sb[:], bg32[:])

        # emb -> bf16 in DRAM
        emb_writes = []
        for i in range(2):
            e32 = prolog.tile([128, 780], f32, tag="e32")
            nc.sync.dma_start(e32[:], flat128(emb_d)[:, i * 780:(i + 1) * 780])
            ebf = prolog.tile([128, 780], bf, tag="ebf")
            nc.vector.tensor_copy(ebf[:], e32[:])
            emb_writes.append(
                nc.sync.dma_start(flat128(emb_bf)[:, i * 780:(i + 1) * 780], ebf[:]))

        # wid -> int16 wrapped-16 index layout, replicated into all 8 groups
        # of 16 partitions (each Q7 SWDGE core reads its own group on HW)
        idx32 = prolog.tile([128, NIDX // 16], mybir.dt.int32, tag="idx32")
        for grp in range(8):
            nc.sync.dma_start(idx32[16 * grp:16 * (grp + 1), :],
                              wid_d.rearrange("(c p) -> p c", p=16))
        idx_cast = nc.vector.tensor_copy(idx16[:], idx32[:])

        for _rep in range(repeat):
            # gather: x[p, fc, i] = emb_bf[wid[i], fc*128 + p]
            # (split into 512-idx pieces: larger single gathers overflow the
            #  HW SWDGE descriptor ring; merged into xt with cheap bf16 copies)
            GC = 512
            for piece in range(NIDX // GC):
                xh = xpool.tile([128, 2, GC], bf, tag="xh")
                g = nc.gpsimd.dma_gather(
                    out_ap=xh[:], in_ap=emb_bf,
                    idxs_ap=idx16[:, piece * (GC // 16):(piece + 1) * (GC // 16)],
                    num_idxs=GC, num_idxs_reg=GC, elem_size=H, transpose=True)
                for wi in emb_writes:
                    add_dep_helper(g.ins, wi.ins, reason="gather after emb_bf write")
                add_dep_helper(g.ins, idx_cast.ins, reason="gather after idx cast")
                for fc in range(2):
                    if fc == 0:
                        nc.vector.tensor_copy(xt[:, fc, piece * GC:(piece + 1) * GC],
                                              xh[:, fc, :])
                    else:
                        nc.scalar.copy(xt[:, fc, piece * GC:(piece + 1) * GC],
                                       xh[:, fc, :])

            xv = [xt[:, fc, 0:NPC].rearrange("p (t n) -> p t n", t=TPC)
                  for fc in range(2)]

            # ---------- level loop ----------
            for lvl, (is_bu, d) in enumerate(LEVELS):
                k = 2 ** d              # edges per tree this level
                K = TPC * k             # total columns
                c0 = 2 ** d - 1         # first child node
                p0 = 2 ** (d - 1) - 1   # first parent node
                a0 = 2 ** d - 2         # m_out 'a' index base (c - 1)
                leaf = is_bu and d == D
                last_td = (not is_bu) and d == D

                if is_bu:
                    m_dst_t, rm_dst_t = mbu, rmbu
                    dst_off = BU_OFF[d]
                elif last_td:
                    m_dst_t, rm_dst_t, dst_off = None, None, 0   # chunk tiles
                else:
                    m_dst_t, rm_dst_t = mtd[d % 2], rmtd[d % 2]
                    dst_off = 0

                # previous-level sources
                if is_bu and not leaf:
                    mp = mbu[:, :, BU_OFF[d + 1]:BU_OFF[d + 1] + 2 * K]
                    rmp = rmbu[:, :, BU_OFF[d + 1]:BU_OFF[d + 1] + 2 * K]
                elif not is_bu:
                    mbu_d = mbu[:, :, BU_OFF[d]:BU_OFF[d] + K]
                    rmbu_d = rmbu[:, :, BU_OFF[d]:BU_OFF[d] + K]
                    if d > 1:
                        mtp = mtd[(d - 1) % 2][:, :, 0:K // 2]
                        rmtp = rmtd[(d - 1) % 2][:, :, 0:K // 2]
                    else:
                        mtp = rmtp = None

                nchunks = (K + CH - 1) // CH
                for cc in range(nchunks):
                    W = min(CH, K - cc * CH)     # chunk width
                    q0 = cc * CH                 # col offset within level
                    ta = q0 // k                 # first tree of chunk
                    nt = W // k                  # trees in chunk

                    # ----- s / rm assembly -----
                    if not leaf:
                        s_t = spool.tile([128, 2, CH], bf, tag="s")
                        rm_t = spool.tile([128, 2, CH], bf, tag="rm")
                        for fc in range(2):
                            if is_bu:
                                # pair-sum of previous level (2 cols -> 1)
                                for src, dst in ((mp, s_t), (rmp, rm_t)):
                                    pv = src[:, fc, 2 * q0:2 * q0 + 2 * W].rearrange(
                                        "p (c two) -> p c two", two=2)
                                    nc.gpsimd.tensor_add(dst[:, fc, 0:W], pv[:, :, 0], pv[:, :, 1])
                            else:
                                half = W // 2
                                for src_td, src_bu, dst in ((mtp, mbu_d, s_t), (rmtp, rmbu_d, rm_t)):
                                    ov = dst[:, fc, 0:W].rearrange("p (c two) -> p c two", two=2)
                                    bv = src_bu[:, fc, q0:q0 + W].rearrange(
                                        "p (c two) -> p c two", two=2)
                                    if d > 1:
                                        tv = src_td[:, fc, q0 // 2:q0 // 2 + half]
                                        nc.gpsimd.tensor_add(ov[:, :, 0], tv, bv[:, :, 1])
                                        nc.gpsimd.tensor_add(ov[:, :, 1], tv, bv[:, :, 0])
                                    else:
                                        nc.gpsimd.tensor_copy(ov[:, :, 0], bv[:, :, 1])
                                        nc.gpsimd.tensor_copy(ov[:, :, 1], bv[:, :, 0])

                    # ----- source/dest x column patterns -----
                    # BU: src = children (contig), dst = parents (dup2)
                    # TD: src = parents (dup2), dst = children (contig)
                    def contig(kc):
                        return xv[kc][:, ta:ta + nt, c0:c0 + k]

                    def dup2(kc):
                        par = xv[kc][:, ta:ta + nt, p0:p0 + max(k // 2, 1)]
                        return par.rearrange("p t (j one) -> p t j one", one=1) \
                                  .broadcast_to([128, nt, max(k // 2, 1), 2])

                    sx = contig if is_bu else dup2
                    dx = dup2 if is_bu else contig

                    # ----- z and h gates -----
                    z_t = zpool.tile([128, 2, CH], bf, tag="z")
                    c_t = zpool.tile([128, 2, CH], bf, tag="c")
                    for w_sb, gate_in, out_t, fn, b_sb in (
                            (wz_sb, s_t if not leaf else None, z_t, AF.Sigmoid, bz_sb),
                            (wh_sb, rm_t if not leaf else None, c_t, AF.Tanh, bh_sb)):
                        for mc in range(2):
                            ps = (ps_z if fn == AF.Sigmoid else ps_h).tile(
                                [128, CH], f32, tag="ps")
                            mm = ps[:, 0:W]
                            for kc in range(2):
                                nc.tensor.matmul(mm, w_sb[:, kc, mc * 128:(mc + 1) * 128],
                                                 sx(kc), start=(kc == 0),
                                                 stop=(kc == 1 and gate_in is None))
                            if gate_in is not None:
                                for kc in range(2):
                                    nc.tensor.matmul(mm, w_sb[:, 2 + kc, mc * 128:(mc + 1) * 128],
                                                     gate_in[:, kc, 0:W],
                                                     start=False, stop=(kc == 1))
                            nc.scalar.activation(out_t[:, mc, 0:W], mm, fn,
                                                 bias=b_sb[:, mc:mc + 1])

                    # ----- m combine -----
                    if last_td:
                        m_cur = tpool.tile([128, 2, CH], bf, tag="mtd7")
                        m_dst = [m_cur[:, fc, 0:W] for fc in range(2)]
                    else:
                        m_dst = [m_dst_t[:, fc, dst_off + q0:dst_off + q0 + W]
                                 for fc in range(2)]
                    for fc in range(2):
                        if leaf:
                            nc.vector.tensor_mul(m_dst[fc], z_t[:, fc, 0:W], c_t[:, fc, 0:W])
                        else:
                            t1 = tpool.tile([128, CH], bf, tag="t1")
                            nc.vector.tensor_sub(t1[:, 0:W], c_t[:, fc, 0:W], s_t[:, fc, 0:W])
                            nc.vector.tensor_mul(t1[:, 0:W], z_t[:, fc, 0:W], t1[:, 0:W])
                            nc.vector.tensor_add(m_dst[fc], t1[:, 0:W], s_t[:, fc, 0:W])

                    # ----- r gate and r*m -----
                    if not last_td:
                        r_t = zpool.tile([128, 2, CH], bf, tag="r")
                        for mc in range(2):
                            ps = ps_r.tile([128, CH], f32, tag="ps")
                            mm = ps[:, 0:W]
                            for kc in range(2):
                                nc.tensor.matmul(mm, wr_sb[:, kc, mc * 128:(mc + 1) * 128],
                                                 dx(kc), start=(kc == 0), stop=False)
                            for kc in range(2):
                                nc.tensor.matmul(mm, ur_sb[:, kc, mc * 128:(mc + 1) * 128],
                                                 m_dst[kc], start=False, stop=(kc == 1))
                            nc.scalar.activation(r_t[:, mc, 0:W], mm, AF.Sigmoid,
                                                 bias=br_sb[:, mc:mc + 1])
                        rm_dst = ([rm_dst_t[:, fc, dst_off + q0:dst_off + q0 + W]
                                   for fc in range(2)] if rm_dst_t is not None else
                                  [tpool.tile([128, CH], bf, tag="rmx")[:, 0:W]
                                   for _ in range(2)])
                        for fc in range(2):
                            nc.vector.tensor_mul(rm_dst[fc], r_t[:, fc, 0:W], m_dst[fc])

                    # ----- m output: transpose + evac + DMA -----
                    nblk = (W + 127) // 128
                    for b in range(nblk):
                        bw = min(128, W - b * 128)
                        pst = ps_t.tile([128, 256], bf, tag="pt")
                        for fc in range(2):
                            nc.tensor.transpose(pst[0:bw, fc * 128:(fc + 1) * 128],
                                                m_dst[fc][:, b * 128:b * 128 + bw], idn[:])
                        st = stg.tile([128, 256], f32, tag="st")
                        if (b % 2) == 0:
                            nc.vector.tensor_copy(st[0:bw, :], pst[0:bw, :])
                        else:
                            nc.scalar.copy(st[0:bw, :], pst[0:bw, :])
                        tb = ta + (b * 128) // k
                        ntb = max(bw // k, 1)
                        bsel = 1 if is_bu else 0
                        dst = mview[tb:tb + ntb, a0:a0 + k, bsel, :]
                        nc.sync.dma_start(dst, st[0:bw, :])

                # ----- root vectors (after BU d=1) -----
                if is_bu and d == 1:
                    acc = spool.tile([128, 2, TPC], bf, tag="acc")
                    mbu1 = mbu[:, :, BU_OFF[1]:BU_OFF[1] + 2 * TPC]
                    for fc in range(2):
                        pv = mbu1[:, fc, :].rearrange("p (c two) -> p c two", two=2)
                        nc.vector.tensor_add(acc[:, fc, :], pv[:, :, 0], pv[:, :, 1])
                    psr = ps_z.tile([TPC, H], f32, tag="ps")
                    for kc in range(2):
                        nc.tensor.matmul(psr[:], xv[kc][:, :, 0:1], wg_sb[:, kc, :],
                                         start=(kc == 0), stop=False)
                    for kc in range(2):
                        nc.tensor.matmul(psr[:], acc[:, kc, :], wg_sb[:, 2 + kc, :],
                                         start=False, stop=False)
                    nc.tensor.matmul(psr[:], ones_sb[:], wgb_sb[:], start=False, stop=True)
                    rt = stg.tile([TPC, H], f32, tag="root")
                    nc.scalar.activation(rt[:], psr[:], AF.Relu)
                    nc.sync.dma_start(root_out, rt[:])

    nc.compile()
    return nc


_CACHE = {}


def _get_program(repeat=1):
    if repeat not in _CACHE:
        _CACHE[repeat] = build_program(repeat)
    return _CACHE[repeat]


def make_in_maps(inputs):
    """Shard FULL inputs into per-core in_maps."""
    gi = {k: np.asarray(v) for k, v in inputs.items()
          if k in ("emb", "Wz_w", "Wz_b", "Wr_w", "Ur_w", "Ur_b",
                   "Wh_w", "Wh_b", "Wg_w", "Wg_b", "wid")}
    wid = gi.pop("wid").astype(np.int32)
    shared = {k: np.ascontiguousarray(v, dtype=np.float32) for k, v in gi.items()}
    in_maps = []
    for c in range(NCORES):
        wid_pad = np.zeros(NIDX, np.int32)
        wid_pad[:NPC] = wid[c * NPC:(c + 1) * NPC]
        m = dict(shared)
        m["wid_pad"] = wid_pad
        in_maps.append(m)
    return in_maps


def assemble(results):
    m = np.concatenate([results[c]["m_out"] for c in range(NCORES)], axis=0)
    roots = np.concatenate([results[c]["root_out"] for c in range(NCORES)], axis=0)
    return m, roots


def kernel(**inputs):
    nc = _get_program(1)
    in_maps = make_in_maps(inputs)
    res = run_bass_kernel_spmd(nc, in_maps, list(range(NCORES)))
    return assemble(res.results)
